# revision 2
# baseline (speedup 1.0000x reference)
"""Trainium2 Bass kernel for 8-head MultiHeadAttention (B=2, S=4096, E=512).

Sharding: 8 cores = 2 batches x 4 query-row chunks of 1024; each core runs
all 8 heads for its (batch, q-range) as 4 head-pairs x 2 query-windows, with
the k-dimension processed in 32 chunks of 128.

v2 design (vs v0):
- K-projection is folded into Q on the host: G = Wq^T @ Wk, so
  scores = (x_q G) . x_k^T and the raw (f16) K tiles are the score matmul's
  stationary operand directly. Only a tiny Q-side projection remains
  (1 blockdiag matmul per 512-col chunk). Wv stays folded into Wo.
- Whole 16-bit pipeline is f16 (x, qp, pt, V, ctx, Wo) - f16's 10-bit
  mantissa keeps the base quantization error ~6e-4, leaving the error
  budget to the Schraudolph trick.
- exp via f16-bits Schraudolph on DVE *and* Pool (both support
  scalar_tensor_tensor): ONE op computes i16 = round((s + 82.93) *
  (184.5 * mask)) whose bits ARE the f16 weights; mask folded in as the
  multiplicand. 22 of 32 k-chunks go this way; 10 use the ACT table exp
  (f16 out) with a post-exp f16 mask multiply on Pool/DVE.
- PE only does matmuls: scores (f16, 512 rows), attention-value flip
  (pt stationary, V+ones moving, 65 rows), q-proj, transposes, out-proj.
  No mask work on PE.
- cx accumulators bank-aligned in PSUM; AV runs 8 half-units behind
  scores; transposes/out-proj trickle into later attention blocks.
"""
import sys
for _p in ('/root/.axon_site/_ro/trn_rl_repo', '/opt/trn_rl_repo'):
    if _p not in sys.path:
        sys.path.append(_p)

import numpy as np
import ml_dtypes

import concourse.bass as bass
import concourse.tile as tile
from concourse import bacc, mybir
from concourse import bass_utils

F32 = mybir.dt.float32
F16 = mybir.dt.float16
I16 = mybir.dt.int16
AF = mybir.ActivationFunctionType
ALU = mybir.AluOpType

N_CORES = 8
B, S, E, H, DH = 2, 4096, 512, 8, 64
QLEN = S // 4           # 1024 q rows per core
KC = S // 128           # 32 k chunks

# f16-bits Schraudolph: i16 = round((s + BB) * (A16 * mask)); the i16 bit
# pattern read as f16 is ~exp(s/8). A16 = 1024*log2(e)/8 rounded to an
# f16-exact value; BB calibrated so the mean weight ratio vs exact exp is 1.
A16 = 184.5
BB = 82.932640

# per-kc class: 'saw' = Schraudolph stt (mask folded, on DVE or Pool);
# 'act' = ACT table exp (f16) + post-exp mask multiply (DVE or Pool).
ACT_KCS = {1, 4, 7, 10, 13, 16, 19, 22, 25, 28}
CLS = {kc: ('act' if kc in ACT_KCS else 'saw') for kc in range(KC)}

# engine cycles: stt for 'saw' tiles Pool:DVE = 3:2; act-mask mult 5:3
STT_CYCLE = ('p', 'p', 'p', 'd', 'd')
MSK_CYCLE = ('p', 'p', 'p', 'p', 'p', 'd', 'd', 'd')

_CACHE = {}


def _build_module():
    nc = bacc.Bacc("TRN2", target_bir_lowering=False, debug=False,
                   enable_asserts=True, num_devices=N_CORES)

    xkT = nc.dram_tensor("xkT", [E, S], F16, kind="ExternalInput").ap()
    xqT = nc.dram_tensor("xqT", [E, QLEN], F16, kind="ExternalInput").ap()
    valp = nc.dram_tensor("valp", [S, H * 65], F16, kind="ExternalInput").ap()
    mbx = nc.dram_tensor("mbx", [S, QLEN], F16, kind="ExternalInput").ap()
    g2 = nc.dram_tensor("g2", [128, 128], F16, kind="ExternalInput").ap()
    woe = nc.dram_tensor("woe", [E, E], F16, kind="ExternalInput").ap()
    bo_b = nc.dram_tensor("bo_b", [128, E], F32, kind="ExternalInput").ap()
    eye_d = nc.dram_tensor("eye", [128, 128], F16, kind="ExternalInput").ap()
    out = nc.dram_tensor("out", [QLEN, E], F32, kind="ExternalOutput").ap()

    with tile.TileContext(nc) as tc:
        _emit(tc, nc, xkT, xqT, valp, mbx, g2, woe, bo_b, eye_d, out)

    nc.compile()
    return nc


def _emit(tc, nc, xkT, xqT, valp, mbx, g2, woe, bo_b, eye_d, out):
    from contextlib import ExitStack
    ctx = ExitStack()
    const = ctx.enter_context(tc.tile_pool(name="const", bufs=1))
    kqp = ctx.enter_context(tc.tile_pool(name="kqp", bufs=1))
    xst = ctx.enter_context(tc.tile_pool(name="xst", bufs=2))
    ptp = ctx.enter_context(tc.tile_pool(name="pt", bufs=12))
    ctn_p = ctx.enter_context(tc.tile_pool(name="ctn", bufs=20))
    osb_p = ctx.enter_context(tc.tile_pool(name="osb", bufs=4))
    psp = ctx.enter_context(tc.tile_pool(name="psp", bufs=6, space="PSUM"))
    ctxp = ctx.enter_context(tc.tile_pool(name="ctxp", bufs=1, space="PSUM"))

    # ---------------- constants ----------------
    g2_sb = const.tile([128, 128], F16, tag="g2")
    nc.sync.dma_start(g2_sb, g2)
    eye = const.tile([128, 128], F16, tag="eye")
    nc.sync.dma_start(eye, eye_d)
    woe_sb = [const.tile([128, E], F16, tag=f"woe{pc}", name=f"woe{pc}")
              for pc in range(4)]
    bo_sb = const.tile([128, E], F32, tag="bo")
    biasB = const.tile([128, 1], F32, tag="biasB")
    nc.vector.memset(biasB, 0.0)

    def load_late_consts():
        for pc in range(4):
            nc.sync.dma_start(woe_sb[pc], woe[pc * 128:(pc + 1) * 128, :])
        nc.sync.dma_start(bo_sb, bo_b)

    # resident masks + V (loaded once, streamed in kc order)
    mbx_res = {c: const.tile([128, QLEN], F16, tag=f"mq{c}", name=f"mq{c}")
               for c in range(KC)}
    valp_t = [const.tile([128, H * 65], F16, tag=f"vp{c}", name=f"vp{c}")
              for c in range(KC)]

    def load_kv_masks(lo, hi):
        for c in range(lo, hi):
            nc.sync.dma_start(mbx_res[c], mbx[c * 128:(c + 1) * 128, :])
            nc.sync.dma_start(valp_t[c], valp[c * 128:(c + 1) * 128, :])

    # raw K tiles double as the score stationary operand; q projections
    qp2 = [kqp.tile([128, QLEN], F16, tag=f"qp2_{p}", name=f"qp2_{p}")
           for p in range(4)]
    concatT = [const.tile([128, QLEN], F16, tag=f"ct{p}", name=f"ct{p}")
               for p in range(4)]

    xs = {}

    def proj_load(pair):
        # two half-tiles so early score chunks start after 0.5MB, not 1MB
        xk0 = xst.tile([128, S // 2], F16, tag="xka", name=f"xka{pair}")
        nc.gpsimd.dma_start(xk0, xkT[pair * 128:(pair + 1) * 128, 0:S // 2])
        xq = xst.tile([128, QLEN], F16, tag="xq", name=f"xq{pair}")
        nc.gpsimd.dma_start(xq, xqT[pair * 128:(pair + 1) * 128, :])
        xk1 = xst.tile([128, S // 2], F16, tag="xkb", name=f"xkb{pair}")
        nc.gpsimd.dma_start(xk1, xkT[pair * 128:(pair + 1) * 128, S // 2:])
        xs[pair] = ((xk0, xk1), xq)

    def proj_chunks(pair):
        """Closures: per 512-col chunk: 1 blockdiag matmul + 1 ACT copy."""
        _, xq = xs[pair]
        works = []

        def chunk(c):
            def run():
                ps = psp.tile([128, 512], F32, tag="ps", name=f"q{pair}_{c}")
                nc.tensor.matmul(ps, lhsT=g2_sb,
                                 rhs=xq[:, c * 512:(c + 1) * 512],
                                 start=True, stop=True)
                nc.scalar.copy(qp2[pair][:, c * 512:(c + 1) * 512], ps)
            return [run]

        for c in range(2):
            works += chunk(c)
        return works

    # per-tile engine pick counters
    stt_i = [0]
    msk_i = [0]

    def stt_engine():
        e = STT_CYCLE[stt_i[0] % len(STT_CYCLE)]
        stt_i[0] += 1
        return nc.gpsimd if e == 'p' else nc.vector

    def msk_engine():
        e = MSK_CYCLE[msk_i[0] % len(MSK_CYCLE)]
        msk_i[0] += 1
        return nc.gpsimd if e == 'p' else nc.vector

    # ---------------- attention ----------------
    def attn(pair, qw, trickle=(), last=False):
        trickle = list(trickle)
        (xk0, xk1), _ = xs[pair]
        cx = ctxp.tile([128, 1024], F32, tag="cx", name=f"cx{pair}_{qw}")

        def scores(kc, h2):
            ps = psp.tile([128, 512], F32, tag="ps",
                          name=f"ps{pair}_{qw}_{kc}_{h2}")
            xk = xk0 if kc < KC // 2 else xk1
            koff = 0 if kc < KC // 2 else S // 2
            nc.tensor.matmul(ps, lhsT=xk[h2 * 64:(h2 + 1) * 64,
                                         kc * 128 - koff:(kc + 1) * 128 - koff],
                             rhs=qp2[pair][h2 * 64:(h2 + 1) * 64,
                                           qw * 512:(qw + 1) * 512],
                             start=True, stop=True)
            return ps

        def expmask(kc, h2, ps):
            ms = mbx_res[kc][:, qw * 512:(qw + 1) * 512]
            if CLS[kc] == 'saw':
                pti = ptp.tile([128, 512], I16, tag="pt",
                               name=f"pt{pair}_{qw}_{kc}_{h2}")
                stt_engine().scalar_tensor_tensor(pti, ps, BB, ms,
                                                  ALU.add, ALU.mult)
                return pti.bitcast(F16)
            pt = ptp.tile([128, 512], F16, tag="pt",
                          name=f"pt{pair}_{qw}_{kc}_{h2}")
            nc.scalar.activation(pt, ps, AF.Exp, bias=biasB, scale=0.125)
            msk_engine().tensor_mul(pt, pt, ms)
            return pt

        def av(kc, h2, pt):
            h = 2 * pair + h2
            # blocks live bank-aligned at h2*512 + qt*65; a matmul's
            # start=True zeroes the full 2KB psum region, so ONLY the first
            # block of each bank may set it.
            for qt in range(4):
                base = h2 * 512 + qt * 65
                nc.tensor.matmul(
                    cx[:, base:base + 65],
                    lhsT=pt[:, qt * 128:qt * 128 + 128],
                    rhs=valp_t[kc][:, h * 65:(h + 1) * 65],
                    start=(kc == 0 and qt == 0), stop=(kc == KC - 1),
                    skip_group_check=True)

        # software-pipelined half-units (one head each); AV lags behind
        # so its pt dependency is long satisfied at PE dispatch time.
        from collections import deque
        pend = deque()
        hu = 0
        for kc in range(KC):
            for h2 in range(2):
                ps = scores(kc, h2)
                lag = 8 if kc < KC - 5 else 4
                if len(pend) >= lag:
                    av(*pend.popleft())
                    if len(pend) >= lag:
                        av(*pend.popleft())
                pt = expmask(kc, h2, ps)
                pend.append((kc, h2, pt))
                if trickle and hu >= 2 and hu % 2 == 0:
                    trickle.pop(0)()
                hu += 1
        # tail: h0's normalize chain starts while h1's last AVs run
        ctn = {}

        def tail_head(h2):
            r = ctn_p.tile([128, 4], F32, tag="rec", name=f"rc{pair}_{qw}_{h2}")
            dn = bass.AP(tensor=cx.tensor, offset=cx.offset + h2 * 512 + 64,
                         ap=[cx.ap[0], [65, 4]])
            with nc.allow_low_precision(reason="softmax denom reciprocal f32"):
                nc.vector.reciprocal(r, dn)
            for qt in range(4):
                t = ctn_p.tile([128, 64], F16, tag="ctn",
                               name=f"cn{pair}_{qw}_{h2}_{qt}")
                nc.vector.tensor_scalar(
                    t, cx[:, h2 * 512 + qt * 65:h2 * 512 + qt * 65 + 64],
                    r[:, qt:qt + 1], None, ALU.mult)
                ctn[(h2, qt)] = t

        while pend:
            kc_, h2_, pt_ = pend.popleft()
            av(kc_, h2_, pt_)
            if kc_ == KC - 1:
                tail_head(h2_)
        for work in trickle:
            work()

        def transp(h2, qt):
            def go():
                tf = psp.tile([128, 512], F32, tag="ps",
                              name=f"tp{pair}_{qw}_{h2}_{qt}")
                tp = tf[0:64, 0:64].bitcast(F16)
                nc.tensor.transpose(tp, ctn[(h2, qt)], eye)
                dst = concatT[pair][h2 * 64:(h2 + 1) * 64,
                                    qw * 512 + qt * 128:qw * 512 + qt * 128 + 128]
                nc.scalar.copy(dst, tp)
            return go

        return [transp(h2, qt) for qt in range(4) for h2 in range(2)]

    def outproj(qts):
        def one(qt):
            def go():
                op = psp.tile([128, 512], F32, tag="ps", name=f"op{qt}")
                for pc in range(4):
                    nc.tensor.matmul(op,
                                     lhsT=concatT[pc][:, qt * 128:(qt + 1) * 128],
                                     rhs=woe_sb[pc],
                                     start=(pc == 0), stop=(pc == 3))
                osb = osb_p.tile([128, E], F32, tag="osb", name=f"osb{qt}")
                nc.vector.scalar_tensor_tensor(osb, op, 1.0, bo_sb,
                                               ALU.mult, ALU.add)
                nc.sync.dma_start(out[qt * 128:(qt + 1) * 128, :], osb)
            return go
        return [one(qt) for qt in qts]

    # ---------------- schedule ----------------
    proj_load(0)
    load_kv_masks(0, 8)
    for work in proj_chunks(0):
        work()
    proj_load(1)
    load_kv_masks(8, KC)
    load_late_consts()
    t00 = attn(0, 0, trickle=proj_chunks(1))
    proj_load(2)
    t01 = attn(0, 1, trickle=t00 + proj_chunks(2))
    t10 = attn(1, 0, trickle=t01)
    proj_load(3)
    t11 = attn(1, 1, trickle=t10 + proj_chunks(3))
    t20 = attn(2, 0, trickle=t11)
    t21 = attn(2, 1, trickle=t20)
    t30 = attn(3, 0, trickle=t21)
    t31 = attn(3, 1, trickle=t30 + outproj(range(4)), last=True)
    ops = outproj(range(4, 8))
    for qt in range(4):
        t31[2 * qt]()      # transp (h2=0, qt)
        t31[2 * qt + 1]()  # transp (h2=1, qt)
        ops[qt]()

    ctx.close()


def _prep_inputs(key, query, value, mask, Wq, Wk, Wv, Wo, bo):
    f16 = ml_dtypes.float16 if hasattr(ml_dtypes, 'float16') else np.float16
    f16 = np.float16
    key = np.asarray(key, np.float32)
    query = np.asarray(query, np.float32)
    value = np.asarray(value, np.float32)
    mask = np.asarray(mask)
    Wq = np.asarray(Wq, np.float32)
    Wk = np.asarray(Wk, np.float32)
    Wv = np.asarray(Wv, np.float32)
    Wo = np.asarray(Wo, np.float32)
    bo = np.asarray(bo, np.float32)

    # Wo_eff[e, h*64+u] = sum_d Wo[e, h*64+d] * Wv[d, u]
    wo_eff = np.empty((E, E), np.float32)
    for h in range(H):
        wo_eff[:, h * DH:(h + 1) * DH] = Wo[:, h * DH:(h + 1) * DH] @ Wv
    woe = np.ascontiguousarray(wo_eff.T).astype(f16)   # [(h,u), e]

    # G = Wq^T @ Wk folded q-side projection; blockdiag over the head pair
    G = (Wq.T @ Wk).astype(np.float32)
    g2 = np.zeros((128, 128), np.float32)
    g2[0:64, 0:64] = G
    g2[64:128, 64:128] = G

    m01 = (mask[0, 0] != 0).astype(np.float32).T  # [k, q] in {0,1}
    # mbx rows: saw chunks scaled by A16, act chunks raw {0,1}
    rowscale = np.empty((S, 1), np.float32)
    for kc in range(KC):
        rowscale[kc * 128:(kc + 1) * 128] = A16 if CLS[kc] == 'saw' else 1.0

    common = {
        "g2": g2.astype(f16),
        "woe": woe,
        "bo_b": np.ascontiguousarray(np.broadcast_to(bo, (128, E))).astype(np.float32),
        "eye": np.eye(128, dtype=np.float32).astype(f16),
    }
    per_b = {}
    for b in range(B):
        vp = np.ones((S, H, 65), np.float32)
        vp[:, :, :64] = value[b].reshape(S, H, DH)
        per_b[b] = {
            "xkT": np.ascontiguousarray(key[b].T).astype(f16),
            "valp": np.ascontiguousarray(vp.reshape(S, H * 65)).astype(f16),
            "qT": query[b].T,
        }
    in_maps = []
    for c in range(N_CORES):
        b, qs = c // 4, (c % 4) * QLEN
        msl = np.ascontiguousarray(m01[:, qs:qs + QLEN])
        in_maps.append({
            "xkT": per_b[b]["xkT"],
            "xqT": np.ascontiguousarray(per_b[b]["qT"][:, qs:qs + QLEN]).astype(f16),
            "valp": per_b[b]["valp"],
            "mbx": (msl * rowscale).astype(f16),
            **common,
        })
    return in_maps


def get_module():
    if "nc" not in _CACHE:
        _CACHE["nc"] = _build_module()
    return _CACHE["nc"]


def kernel(key, query, value, mask, Wq, Wk, Wv, Wo, bo, **_):
    nc = get_module()
    in_maps = _prep_inputs(key, query, value, mask, Wq, Wk, Wv, Wo, bo)
    res = bass_utils.run_bass_kernel_spmd(
        nc, in_maps, core_ids=list(range(N_CORES)))
    full = np.empty((B, S, E), np.float32)
    for c in range(N_CORES):
        b, qs = c // 4, (c % 4) * QLEN
        full[b, qs:qs + QLEN, :] = res.results[c]["out"]
    return full


# revision 11
# speedup vs baseline: 1.2997x; 1.2997x over previous
"""Trainium2 Bass kernel for 8-head MultiHeadAttention (B=2, S=4096, E=512).

Sharding: 8 cores = 2 batches x 4 query-row chunks of 1024; each core runs
all 8 heads for its (batch, q-range) as 4 head-pairs x 2 query-windows, with
the k-dimension processed in 32 chunks of 128.

v2 design (vs v0):
- K-projection is folded into Q on the host: G = Wq^T @ Wk, so
  scores = (x_q G) . x_k^T and the raw (f16) K tiles are the score matmul's
  stationary operand directly. Only a tiny Q-side projection remains
  (1 blockdiag matmul per 512-col chunk). Wv stays folded into Wo.
- Whole 16-bit pipeline is f16 (x, qp, pt, V, ctx, Wo) - f16's 10-bit
  mantissa keeps the base quantization error ~6e-4, leaving the error
  budget to the Schraudolph trick.
- exp via f16-bits Schraudolph on DVE *and* Pool (both support
  scalar_tensor_tensor): ONE op computes i16 = round((s + 82.93) *
  (184.5 * mask)) whose bits ARE the f16 weights; mask folded in as the
  multiplicand. 22 of 32 k-chunks go this way; 10 use the ACT table exp
  (f16 out) with a post-exp f16 mask multiply on Pool/DVE.
- PE only does matmuls: scores (f16, 512 rows), attention-value flip
  (pt stationary, V+ones moving, 65 rows), q-proj, transposes, out-proj.
  No mask work on PE.
- cx accumulators bank-aligned in PSUM; AV runs 8 half-units behind
  scores; transposes/out-proj trickle into later attention blocks.
"""
import sys
for _p in ('/root/.axon_site/_ro/trn_rl_repo', '/opt/trn_rl_repo'):
    if _p not in sys.path:
        sys.path.append(_p)

import numpy as np
import ml_dtypes

import concourse.bass as bass
import concourse.tile as tile
from concourse import bacc, mybir
from concourse import bass_utils

F32 = mybir.dt.float32
F16 = mybir.dt.float16
I16 = mybir.dt.int16
AF = mybir.ActivationFunctionType
ALU = mybir.AluOpType

N_CORES = 8
B, S, E, H, DH = 2, 4096, 512, 8, 64
QLEN = S // 4           # 1024 q rows per core
KC = S // 128           # 32 k chunks

# f16-bits Schraudolph: i16 = round((s + BB) * (A16 * mask)); the i16 bit
# pattern read as f16 is ~exp(s/8). A16 = 1024*log2(e)/8 rounded to an
# f16-exact value; BB calibrated so the mean weight ratio vs exact exp is 1.
A16 = 184.5
BB = 82.932640

# per-kc class: 'saw' = Schraudolph stt (mask folded, on DVE or Pool);
# 'act' = ACT table exp (f16) + post-exp mask multiply on DVE.
ACT_KCS = set(range(1, KC, 2))   # odd kcs: 16 chunks on ACT exp
CLS = {kc: ('act' if kc in ACT_KCS else 'saw') for kc in range(KC)}

# engine cycles: stt for 'saw' tiles Pool:DVE = 7:3 (Pool runs gpsimd ops at
# 0.6x roofline); act-mask mult always DVE (Pool TT is 0.42x); transpose
# copies alternate ACT/DVE.
STT_CYCLE = ('p', 'p', 'p', 'd', 'p', 'p', 'd', 'p', 'p', 'd')
TCP_CYCLE = ('a', 'd')

_CACHE = {}


def _build_module():
    nc = bacc.Bacc("TRN2", target_bir_lowering=False, debug=False,
                   enable_asserts=True, num_devices=N_CORES)

    xkT = nc.dram_tensor("xkT", [E, S], F16, kind="ExternalInput").ap()
    xqT = nc.dram_tensor("xqT", [E, QLEN], F16, kind="ExternalInput").ap()
    valp = nc.dram_tensor("valp", [S, H * 65], F16, kind="ExternalInput").ap()
    mbx = nc.dram_tensor("mbx", [S, QLEN], F16, kind="ExternalInput").ap()
    g2 = nc.dram_tensor("g2", [128, 128], F16, kind="ExternalInput").ap()
    woe = nc.dram_tensor("woe", [E, E], F16, kind="ExternalInput").ap()
    bo_b = nc.dram_tensor("bo_b", [128, E], F32, kind="ExternalInput").ap()
    eye_d = nc.dram_tensor("eye", [128, 128], F16, kind="ExternalInput").ap()
    out = nc.dram_tensor("out", [QLEN, E], F32, kind="ExternalOutput").ap()

    with tile.TileContext(nc) as tc:
        _emit(tc, nc, xkT, xqT, valp, mbx, g2, woe, bo_b, eye_d, out)

    nc.compile()
    return nc


def _emit(tc, nc, xkT, xqT, valp, mbx, g2, woe, bo_b, eye_d, out):
    from contextlib import ExitStack
    ctx = ExitStack()
    const = ctx.enter_context(tc.tile_pool(name="const", bufs=1))
    kqp = ctx.enter_context(tc.tile_pool(name="kqp", bufs=1))
    xst = ctx.enter_context(tc.tile_pool(name="xst", bufs=2))
    ptp = ctx.enter_context(tc.tile_pool(name="pt", bufs=12))
    ctn_p = ctx.enter_context(tc.tile_pool(name="ctn", bufs=20))
    osb_p = ctx.enter_context(tc.tile_pool(name="osb", bufs=4))
    psp = ctx.enter_context(tc.tile_pool(name="psp", bufs=6, space="PSUM"))
    ctxp = ctx.enter_context(tc.tile_pool(name="ctxp", bufs=1, space="PSUM"))

    # ---------------- constants ----------------
    g2_sb = const.tile([128, 128], F16, tag="g2")
    nc.sync.dma_start(g2_sb, g2)
    eye = const.tile([128, 128], F16, tag="eye")
    nc.sync.dma_start(eye, eye_d)
    woe_sb = [const.tile([128, E], F16, tag=f"woe{pc}", name=f"woe{pc}")
              for pc in range(4)]
    bo_sb = const.tile([128, E], F32, tag="bo")
    biasB = const.tile([128, 1], F32, tag="biasB")
    nc.vector.memset(biasB, 0.0)

    def load_late_consts():
        for pc in range(4):
            nc.sync.dma_start(woe_sb[pc], woe[pc * 128:(pc + 1) * 128, :])
        nc.sync.dma_start(bo_sb, bo_b)

    # resident masks + V (loaded once, streamed in kc order)
    mbx_res = {c: const.tile([128, QLEN], F16, tag=f"mq{c}", name=f"mq{c}")
               for c in range(KC)}
    valp_t = [const.tile([128, H * 65], F16, tag=f"vp{c}", name=f"vp{c}")
              for c in range(KC)]

    def load_kv_masks(lo, hi):
        for c in range(lo, hi):
            nc.sync.dma_start(mbx_res[c], mbx[c * 128:(c + 1) * 128, :])
            nc.sync.dma_start(valp_t[c], valp[c * 128:(c + 1) * 128, :])

    # raw K tiles double as the score stationary operand; q projections
    qp2 = [kqp.tile([128, QLEN], F16, tag=f"qp2_{p}", name=f"qp2_{p}")
           for p in range(4)]
    concatT = [const.tile([128, QLEN], F16, tag=f"ct{p}", name=f"ct{p}")
               for p in range(4)]

    xs = {}

    def proj_load(pair):
        # HWDGE on the ACT queue: no Pool desc-gen cost, and trickle-paced
        # call sites keep these transfers out of the resident-stream window.
        xk0 = xst.tile([128, S // 2], F16, tag="xka", name=f"xka{pair}")
        nc.scalar.dma_start(xk0, xkT[pair * 128:(pair + 1) * 128, 0:S // 2])
        xq = xst.tile([128, QLEN], F16, tag="xq", name=f"xq{pair}")
        nc.scalar.dma_start(xq, xqT[pair * 128:(pair + 1) * 128, :])
        xk1 = xst.tile([128, S // 2], F16, tag="xkb", name=f"xkb{pair}")
        nc.scalar.dma_start(xk1, xkT[pair * 128:(pair + 1) * 128, S // 2:])
        xs[pair] = ((xk0, xk1), xq)

    def proj_chunks(pair):
        """Closures: per 512-col chunk: 1 blockdiag matmul + 1 ACT copy."""
        _, xq = xs[pair]
        works = []

        def chunk(c):
            def run():
                ps = psp.tile([128, 512], F32, tag="ps", name=f"q{pair}_{c}")
                nc.tensor.matmul(ps, lhsT=g2_sb,
                                 rhs=xq[:, c * 512:(c + 1) * 512],
                                 start=True, stop=True)
                nc.scalar.copy(qp2[pair][:, c * 512:(c + 1) * 512], ps)
            return [run]

        for c in range(2):
            works += chunk(c)
        return works

    # per-tile engine pick counters
    stt_i = [0]
    tcp_i = [0]

    def stt_engine():
        e = STT_CYCLE[stt_i[0] % len(STT_CYCLE)]
        stt_i[0] += 1
        return nc.gpsimd if e == 'p' else nc.vector

    # ---------------- attention ----------------
    def attn(pair, qw, trickle=(), last=False):
        trickle = list(trickle)
        (xk0, xk1), _ = xs[pair]
        cx = ctxp.tile([128, 1024], F32, tag="cx", name=f"cx{pair}_{qw}")

        def scores(kc, h2):
            ps = psp.tile([128, 512], F32, tag="ps",
                          name=f"ps{pair}_{qw}_{kc}_{h2}")
            xk = xk0 if kc < KC // 2 else xk1
            koff = 0 if kc < KC // 2 else S // 2
            nc.tensor.matmul(ps, lhsT=xk[h2 * 64:(h2 + 1) * 64,
                                         kc * 128 - koff:(kc + 1) * 128 - koff],
                             rhs=qp2[pair][h2 * 64:(h2 + 1) * 64,
                                           qw * 512:(qw + 1) * 512],
                             start=True, stop=True)
            return ps

        def expmask(kc, h2, ps):
            ms = mbx_res[kc][:, qw * 512:(qw + 1) * 512]
            if CLS[kc] == 'saw':
                pti = ptp.tile([128, 512], I16, tag="pt",
                               name=f"pt{pair}_{qw}_{kc}_{h2}")
                stt_engine().scalar_tensor_tensor(pti, ps, BB, ms,
                                                  ALU.add, ALU.mult)
                return pti.bitcast(F16)
            pt = ptp.tile([128, 512], F16, tag="pt",
                          name=f"pt{pair}_{qw}_{kc}_{h2}")
            nc.scalar.activation(pt, ps, AF.Exp, bias=biasB, scale=0.125)
            nc.vector.tensor_mul(pt, pt, ms)
            return pt

        def av(kc, h2, pt):
            h = 2 * pair + h2
            # blocks live bank-aligned at h2*512 + qt*65; a matmul's
            # start=True zeroes the full 2KB psum region, so ONLY the first
            # block of each bank may set it.
            for qt in range(4):
                base = h2 * 512 + qt * 65
                nc.tensor.matmul(
                    cx[:, base:base + 65],
                    lhsT=pt[:, qt * 128:qt * 128 + 128],
                    rhs=valp_t[kc][:, h * 65:(h + 1) * 65],
                    start=(kc == 0 and qt == 0), stop=(kc == KC - 1),
                    skip_group_check=True)

        # software-pipelined half-units (one head each); AV lags behind
        # so its pt dependency is long satisfied at PE dispatch time.
        from collections import deque
        pend = deque()
        hu = 0
        for kc in range(KC):
            for h2 in range(2):
                ps = scores(kc, h2)
                lag = 8 if kc < KC - 5 else 4
                if len(pend) >= lag:
                    av(*pend.popleft())
                    if len(pend) >= lag:
                        av(*pend.popleft())
                pt = expmask(kc, h2, ps)
                pend.append((kc, h2, pt))
                if trickle and hu >= 2 and hu % 2 == 0:
                    w = trickle.pop(0)
                    if w is not None:
                        w()
                hu += 1
        # tail: h0's normalize chain starts while h1's last AVs run
        ctn = {}

        def tail_head(h2):
            r = ctn_p.tile([128, 4], F32, tag="rec", name=f"rc{pair}_{qw}_{h2}")
            dn = bass.AP(tensor=cx.tensor, offset=cx.offset + h2 * 512 + 64,
                         ap=[cx.ap[0], [65, 4]])
            with nc.allow_low_precision(reason="softmax denom reciprocal f32"):
                nc.vector.reciprocal(r, dn)
            for qt in range(4):
                t = ctn_p.tile([128, 64], F16, tag="ctn",
                               name=f"cn{pair}_{qw}_{h2}_{qt}")
                nc.scalar.activation(
                    t, cx[:, h2 * 512 + qt * 65:h2 * 512 + qt * 65 + 64],
                    AF.Copy, bias=0.0, scale=r[:, qt:qt + 1])
                ctn[(h2, qt)] = t

        while pend:
            kc_, h2_, pt_ = pend.popleft()
            av(kc_, h2_, pt_)
            if kc_ == KC - 1:
                tail_head(h2_)
        for work in trickle:
            if work is not None:
                work()

        def transp(h2, qt):
            def go():
                tf = psp.tile([128, 512], F32, tag="ps",
                              name=f"tp{pair}_{qw}_{h2}_{qt}")
                tp = tf[0:64, 0:64].bitcast(F16)
                nc.tensor.transpose(tp, ctn[(h2, qt)], eye)
                dst = concatT[pair][h2 * 64:(h2 + 1) * 64,
                                    qw * 512 + qt * 128:qw * 512 + qt * 128 + 128]
                e = TCP_CYCLE[tcp_i[0] % len(TCP_CYCLE)]
                tcp_i[0] += 1
                if e == 'a':
                    nc.scalar.copy(dst, tp)
                else:
                    nc.vector.tensor_copy(dst, tp)
            return go

        return [transp(h2, qt) for qt in range(4) for h2 in range(2)]

    def outproj(qts):
        def one(qt):
            def go():
                op = psp.tile([128, 512], F32, tag="ps", name=f"op{qt}")
                for pc in range(4):
                    nc.tensor.matmul(op,
                                     lhsT=concatT[pc][:, qt * 128:(qt + 1) * 128],
                                     rhs=woe_sb[pc],
                                     start=(pc == 0), stop=(pc == 3))
                osb = osb_p.tile([128, E], F32, tag="osb", name=f"osb{qt}")
                nc.vector.scalar_tensor_tensor(osb, op, 1.0, bo_sb,
                                               ALU.mult, ALU.add)
                nc.sync.dma_start(out[qt * 128:(qt + 1) * 128, :], osb)
            return go
        return [one(qt) for qt in qts]

    # ---------------- schedule ----------------
    proj_load(0)
    load_kv_masks(0, 8)
    for work in proj_chunks(0):
        work()
    load_kv_masks(8, KC)
    load_late_consts()
    # pair-1 x loads are trickled LATE into block (0,0) so their DMA traffic
    # stays out of the resident mask/value stream's critical window.
    t00 = attn(0, 0, trickle=[None] * 19 + [lambda: proj_load(1)])
    t01 = attn(0, 1, trickle=t00 + proj_chunks(1))
    proj_load(2)
    t10 = attn(1, 0, trickle=t01 + proj_chunks(2))
    proj_load(3)
    t11 = attn(1, 1, trickle=t10)
    t20 = attn(2, 0, trickle=t11 + proj_chunks(3))
    t21 = attn(2, 1, trickle=t20)
    t30 = attn(3, 0, trickle=t21)
    t31 = attn(3, 1, trickle=t30 + outproj(range(4)), last=True)
    ops = outproj(range(4, 8))
    for qt in range(4):
        t31[2 * qt]()      # transp (h2=0, qt)
        t31[2 * qt + 1]()  # transp (h2=1, qt)
        ops[qt]()

    ctx.close()


def _prep_inputs(key, query, value, mask, Wq, Wk, Wv, Wo, bo):
    f16 = ml_dtypes.float16 if hasattr(ml_dtypes, 'float16') else np.float16
    f16 = np.float16
    key = np.asarray(key, np.float32)
    query = np.asarray(query, np.float32)
    value = np.asarray(value, np.float32)
    mask = np.asarray(mask)
    Wq = np.asarray(Wq, np.float32)
    Wk = np.asarray(Wk, np.float32)
    Wv = np.asarray(Wv, np.float32)
    Wo = np.asarray(Wo, np.float32)
    bo = np.asarray(bo, np.float32)

    # Wo_eff[e, h*64+u] = sum_d Wo[e, h*64+d] * Wv[d, u]
    wo_eff = np.empty((E, E), np.float32)
    for h in range(H):
        wo_eff[:, h * DH:(h + 1) * DH] = Wo[:, h * DH:(h + 1) * DH] @ Wv
    woe = np.ascontiguousarray(wo_eff.T).astype(f16)   # [(h,u), e]

    # G = Wq^T @ Wk folded q-side projection; blockdiag over the head pair
    G = (Wq.T @ Wk).astype(np.float32)
    g2 = np.zeros((128, 128), np.float32)
    g2[0:64, 0:64] = G
    g2[64:128, 64:128] = G

    m01 = (mask[0, 0] != 0).astype(np.float32).T  # [k, q] in {0,1}
    # mbx rows: saw chunks scaled by A16, act chunks raw {0,1}
    rowscale = np.empty((S, 1), np.float32)
    for kc in range(KC):
        rowscale[kc * 128:(kc + 1) * 128] = A16 if CLS[kc] == 'saw' else 1.0

    common = {
        "g2": g2.astype(f16),
        "woe": woe,
        "bo_b": np.ascontiguousarray(np.broadcast_to(bo, (128, E))).astype(np.float32),
        "eye": np.eye(128, dtype=np.float32).astype(f16),
    }
    per_b = {}
    for b in range(B):
        vp = np.ones((S, H, 65), np.float32)
        vp[:, :, :64] = value[b].reshape(S, H, DH)
        per_b[b] = {
            "xkT": np.ascontiguousarray(key[b].T).astype(f16),
            "valp": np.ascontiguousarray(vp.reshape(S, H * 65)).astype(f16),
            "qT": query[b].T,
        }
    in_maps = []
    for c in range(N_CORES):
        b, qs = c // 4, (c % 4) * QLEN
        msl = np.ascontiguousarray(m01[:, qs:qs + QLEN])
        in_maps.append({
            "xkT": per_b[b]["xkT"],
            "xqT": np.ascontiguousarray(per_b[b]["qT"][:, qs:qs + QLEN]).astype(f16),
            "valp": per_b[b]["valp"],
            "mbx": (msl * rowscale).astype(f16),
            **common,
        })
    return in_maps


def get_module():
    if "nc" not in _CACHE:
        _CACHE["nc"] = _build_module()
    return _CACHE["nc"]


def kernel(key, query, value, mask, Wq, Wk, Wv, Wo, bo, **_):
    nc = get_module()
    in_maps = _prep_inputs(key, query, value, mask, Wq, Wk, Wv, Wo, bo)
    res = bass_utils.run_bass_kernel_spmd(
        nc, in_maps, core_ids=list(range(N_CORES)))
    full = np.empty((B, S, E), np.float32)
    for c in range(N_CORES):
        b, qs = c // 4, (c % 4) * QLEN
        full[b, qs:qs + QLEN, :] = res.results[c]["out"]
    return full
